# revision 19
# baseline (speedup 1.0000x reference)
"""Trainium2 Bass kernel for batched multi-head attention (B=2, S=2048, E=1024, H=16).

Sharding: core r = 4*b + g handles batch b and head-group g (4 heads, 256 emb cols).
- QKV projections: tensor-parallel over head groups (each core computes its 256
  output cols from the full 1024-dim input, streamed in 128-row blocks).
- Attention: each core runs 4 heads over all 2048 queries. Scores kept transposed
  [kt, qt]; softmax normalization deferred via a mask/ones column appended to V.
  The two heads of a 128-partition tile are software-pipelined (scores run one
  key-block ahead of PV) so the PE never waits on the Exp activation, and
  K-block weight loads overlap the other head's matmuls (disjoint row groups).
- Out-proj: token-parallel. Contexts are exchanged with per-head 4-rank AllToAll
  within each batch's core group; each core then computes its 512-token slice.
The whole datapath runs in fp16 (full-rate PE streaming + fast weight load,
half DMA bytes); all matmul accumulation stays in fp32 PSUM.
"""

import sys

if '/opt/trn_rl_repo' not in sys.path:
    sys.path.insert(0, '/opt/trn_rl_repo')

import numpy as np

P = 128
B, S, E, H, DH = 2, 2048, 1024, 16, 64
NCORES = 8
G = 4                 # head groups == cores per batch
EG = E // G           # 256 emb cols per group
TS = S // G           # 512 tokens per core in out-proj
KB = S // P           # 16 key-token blocks
IB = E // P           # 8 contraction blocks of 128
QW = 512              # matmul moving free-dim chunk
HW = 1024             # query half width in attention
SCALE = DH ** -0.5

_cache = {}


def _build():
    import concourse.bass as bass
    import concourse.mybir as mybir
    import concourse.tile as tile
    from concourse import bacc
    from contextlib import ExitStack

    f32 = mybir.dt.float32
    f16 = mybir.dt.float16
    AF = mybir.ActivationFunctionType

    nc = bacc.Bacc("TRN2", target_bir_lowering=False, debug=False,
                   num_devices=NCORES)

    xqT = nc.dram_tensor("xqT", [E, S], f16, kind="ExternalInput").ap()
    xkT = nc.dram_tensor("xkT", [E, S], f16, kind="ExternalInput").ap()
    xvT = nc.dram_tensor("xvT", [E, S], f16, kind="ExternalInput").ap()
    wqT = nc.dram_tensor("wqT", [E, EG], f16, kind="ExternalInput").ap()
    wkT = nc.dram_tensor("wkT", [E, EG], f16, kind="ExternalInput").ap()
    wvT = nc.dram_tensor("wvT", [E, EG], f16, kind="ExternalInput").ap()
    woT = nc.dram_tensor("woT", [E, E], f16, kind="ExternalInput").ap()
    # packed constants: [ones(512) | bq(256) | bk(256) | bv(256) | bo(1024)]
    cpack = nc.dram_tensor("cpack", [1, QW + 3 * EG + E], f16,
                           kind="ExternalInput").ap()
    # packed f32 per-key constants: [mask_pb(16) | maskrep(64) | zmask(8)]
    mpack = nc.dram_tensor("mpack", [P, KB + KB * G + NCORES], f32,
                           kind="ExternalInput").ap()
    out = nc.dram_tensor("out", [TS, E], f32, kind="ExternalOutput").ap()

    a2a_ins = [nc.dram_tensor(f"a2a_in{mt}", [NCORES, P, TS], f16).ap()
               for mt in range(2)]
    a2a_outs = [nc.dram_tensor(f"a2a_out{mt}", [NCORES, P, TS], f16).ap()
                for mt in range(2)]
    groups = [list(range(NCORES))]

    with tile.TileContext(nc) as tc, ExitStack() as top:
        const = top.enter_context(tc.tile_pool(name="const", bufs=1))

        cpk = const.tile([1, QW + 3 * EG + E], f16)
        nc.sync.dma_start(cpk[:], cpack[:])
        ones_b = cpk[:, 0:QW]
        bq_r = cpk[:, QW:QW + EG]
        bk_r = cpk[:, QW + EG:QW + 2 * EG]
        bv_r = cpk[:, QW + 2 * EG:QW + 3 * EG]
        bo_r = cpk[:, QW + 3 * EG:QW + 3 * EG + E]
        mpk = const.tile([P, KB + KB * G + NCORES], f32)
        nc.sync.dma_start(mpk[:], mpack[:])
        mask_t = mpk[:, 0:KB]
        maskrep_t = mpk[:, KB:KB + KB * G]
        zmask_t = mpk[:, KB + KB * G:]

        # persistent projection outputs
        proj_sb = top.enter_context(tc.tile_pool(name="proj_sb", bufs=1))
        qpT = [proj_sb.tile([P, S], f16, tag=f"qpT{m}", name=f"qpT{m}")
               for m in range(2)]
        kpT = [proj_sb.tile([P, S], f16, tag=f"kpT{m}", name=f"kpT{m}")
               for m in range(2)]
        # vp tiles: per kt-block, [P, 4 heads x (64 vals + 1 mask col)]
        vp_sb = [proj_sb.tile([P, G * (DH + 1)], f16, tag=f"vp{m}", name=f"vp{m}")
                 for m in range(KB)]

        # ---- projection weights ----
        with tc.tile_pool(name="wqkv", bufs=1) as wqkv:
            w_r = {}
            for name, wap in (("v", wvT), ("k", wkT), ("q", wqT)):
                wr = wqkv.tile([P, IB * EG], f16, tag=f"w{name}r", name=f"w{name}r")
                nc.sync.dma_start(
                    wr.rearrange("p (i c) -> p i c", c=EG),
                    wap.rearrange("(i p) c -> p i c", p=P))
                w_r[name] = wr

            # ---- V projection, token-major: out [t, e] directly ----
            # stationary = x block [128 i-dims, 128 tokens], moving = Wv
            # [128 i-dims, 256 cols]; accumulate over the 8 i-blocks, add
            # bias, then mask-scale straight from PSUM into vp_sb.
            with tc.tile_pool(name="xall", bufs=1) as xall_p:
                xall = []
                for i in range(IB):
                    xr = xall_p.tile([P, S], f16, tag=f"xv{i}", name=f"xv{i}")
                    for c in range(S // QW):
                        nc.sync.dma_start(
                            xr[:, c * QW:(c + 1) * QW],
                            xvT[i * P:(i + 1) * P, c * QW:(c + 1) * QW])
                    xall.append(xr)
                with tc.tile_pool(name="vpsum", bufs=4, space="PSUM") as vpsum:
                    for t in range(KB):
                        vps = vpsum.tile([P, EG], f32)
                        for i in range(IB):
                            nc.tensor.matmul(
                                vps[:], xall[i][:, t * P:(t + 1) * P],
                                w_r["v"][:, i * EG:(i + 1) * EG],
                                start=(i == 0), stop=False)
                        nc.tensor.matmul(
                            vps[:], ones_b[:, 0:P], bv_r[:],
                            start=False, stop=True)
                        dst3 = vp_sb[t].rearrange("p (h e) -> p h e", e=DH + 1)
                        nc.vector.tensor_scalar_mul(
                            dst3[:, :, 0:DH],
                            vps.rearrange("p (h e) -> p h e", e=DH),
                            mask_t[:, t:t + 1])
                        nc.vector.tensor_copy(
                            dst3[:, :, DH:DH + 1],
                            maskrep_t[:, t * G:(t + 1) * G]
                            .rearrange("p (h e) -> p h e", e=1))

            # ---- K and Q projections: out [e_sel, t] transposed ----
            with tc.tile_pool(name="xst", bufs=4) as xst, \
                 tc.tile_pool(name="kqpsum", bufs=1, space="PSUM") as kqpsum:
                for name, xap, brow, dsts in (("k", xkT, bk_r, kpT),
                                              ("q", xqT, bq_r, qpT)):
                    kqs = [kqpsum.tile([P, S], f32, tag=f"kqs{m}", name=f"kqs{m}")
                           for m in range(2)]
                    for i in range(IB):
                        xr = xst.tile([P, S], f16)
                        for c in range(S // QW):
                            nc.sync.dma_start(
                                xr[:, c * QW:(c + 1) * QW],
                                xap[i * P:(i + 1) * P, c * QW:(c + 1) * QW])
                        for m in range(2):
                            for c in range(S // QW):
                                nc.tensor.matmul(
                                    kqs[m][:, c * QW:(c + 1) * QW],
                                    w_r[name][:, i * EG + m * P:i * EG + (m + 1) * P],
                                    xr[:, c * QW:(c + 1) * QW],
                                    start=(i == 0), stop=False)
                    for m in range(2):
                        for c in range(S // QW):
                            nc.tensor.matmul(
                                kqs[m][:, c * QW:(c + 1) * QW],
                                brow[:, m * P:(m + 1) * P],
                                ones_b[:],
                                start=False, stop=True)
                    for m in range(2):
                        nc.vector.tensor_copy(dsts[m][:], kqs[m][:])

        # ---- out-proj weights: loaded during attention ----
        wo_pool = top.enter_context(tc.tile_pool(name="wo", bufs=1))
        wo_r = wo_pool.tile([P, IB * E], f16)
        for i in range(IB):
            for c in range(2):
                nc.sync.dma_start(
                    wo_r[c * 64:(c + 1) * 64, i * E:(i + 1) * E],
                    woT[i * P + c * 64:i * P + (c + 1) * 64, :])

        # ga[ib] holds emb rows [ib*128, (ib+1)*128) of the concat context =
        # head pair ib%2 of group ib//2; gathered right after that pair's
        # AllToAll so the DMAs overlap the remaining attention work.
        gap = top.enter_context(tc.tile_pool(name="gap", bufs=1))
        gstage = top.enter_context(tc.tile_pool(name="gstage", bufs=4))
        ga = {}

        # ---- attention: head pair (2mt, 2mt+1), scores one j-block ahead ----
        # PSUM budget: sp0/sp1 [128,1024] (2 banks each) + pv0/pv1 [65,1024]
        # (2 banks each) = 8 banks, all at bufs=1.
        with tc.tile_pool(name="spsum", bufs=1, space="PSUM") as spsum, \
             tc.tile_pool(name="pvpsum", bufs=1, space="PSUM") as pvpsum, \
             tc.tile_pool(name="expp", bufs=3) as expp, \
             tc.tile_pool(name="normp", bufs=2) as normp, \
             tc.tile_pool(name="sendp", bufs=3) as sendp:
            for mt in range(2):
                for half in range(2):
                    q0 = half * HW
                    pv = [pvpsum.tile([DH + 1, HW], f32, tag=f"pv{hh}",
                                      name=f"pv{hh}") for hh in range(2)]
                    es_cur = [None, None]
                    es_nxt = [None, None]

                    def emit_scores(j, es_dst):
                        for hh in range(2):
                            po = hh * DH
                            sp = spsum.tile([P, HW], f32, tag=f"sp{hh}",
                                            name=f"sp{hh}")
                            for c in range(HW // QW):
                                nc.tensor.matmul(
                                    sp[:, c * QW:(c + 1) * QW],
                                    kpT[mt][po:po + DH, j * P:(j + 1) * P],
                                    qpT[mt][po:po + DH,
                                            q0 + c * QW:q0 + (c + 1) * QW],
                                    start=True, stop=True)
                            es = expp.tile([P, HW], f16, tag=f"es{hh}",
                                           name=f"es{hh}")
                            nc.scalar.activation(es[:], sp[:], AF.Exp,
                                                 scale=SCALE)
                            es_dst[hh] = es

                    emit_scores(0, es_cur)
                    for j in range(KB):
                        if j + 1 < KB:
                            emit_scores(j + 1, es_nxt)
                        for hh in range(2):
                            h = 2 * mt + hh
                            for c in range(HW // QW):
                                nc.tensor.matmul(
                                    pv[hh][:, c * QW:(c + 1) * QW],
                                    vp_sb[j][:, h * (DH + 1):(h + 1) * (DH + 1)],
                                    es_cur[hh][:, c * QW:(c + 1) * QW],
                                    start=(j == 0), stop=(j == KB - 1))
                        es_cur, es_nxt = es_nxt, es_cur

                    # normalize. The reciprocal runs on the ACT engine
                    # (table lookup, ~1us, plenty accurate for 2e-2) so the
                    # slow DVE reciprocal never gates pv reuse or the sends.
                    for hh in range(2):
                        h = 2 * mt + hh
                        rec = normp.tile([1, HW], f32, tag=f"rec{hh}",
                                         name=f"rec{hh}")
                        nc.scalar.add_instruction(mybir.InstActivation(
                            name=nc.get_next_instruction_name(),
                            func=AF.Reciprocal,
                            ins=[nc.scalar.lower_ap(pv[hh][DH:DH + 1, :]),
                                 mybir.ImmediateValue(dtype=f32, value=0.0),
                                 mybir.ImmediateValue(dtype=f32, value=1.0),
                                 mybir.ImmediateValue(dtype=f32, value=0.0)],
                            outs=[nc.scalar.lower_ap(rec[:])]))
                        recB = normp.tile([DH, HW], f32, tag=f"recB{hh}",
                                          name=f"recB{hh}")
                        nc.gpsimd.partition_broadcast(recB[:], rec[:])
                        ctxn = normp.tile([DH, HW], f16, tag=f"ctxn{hh}",
                                          name=f"ctxn{hh}")
                        nc.vector.tensor_mul(ctxn[:], pv[hh][0:DH, :], recB[:])
                        # slots: my-batch pair (s, s+4); zmask zeroes the
                        # cross-batch one so the receiver's pair-sum works
                        for jj in range(2):
                            shard = half * 2 + jj
                            for slot in (shard, shard + 4):
                                st = sendp.tile([DH, TS], f16)
                                nc.vector.tensor_scalar_mul(
                                    st[:], ctxn[:, jj * TS:(jj + 1) * TS],
                                    zmask_t[0:DH, slot:slot + 1])
                                nc.sync.dma_start(
                                    a2a_ins[mt][slot][hh * DH:(hh + 1) * DH, :],
                                    st[:])
                # one exchange for the whole pair (both heads stacked in the
                # slot payload, matching the ga[ib] row layout exactly)
                nc.gpsimd.collective_compute(
                    "AllToAll", mybir.AluOpType.bypass,
                    replica_groups=groups,
                    ins=[a2a_ins[mt][:]], outs=[a2a_outs[mt][:]])
                if mt == 1:
                    # gather both pairs now: pair-0's exchange finished long
                    # ago, and emitting gathers only after the final
                    # collective keeps its send descriptors unblocked
                    for gmt in range(2):
                        for gp in range(G):
                            ib = gp * 2 + gmt
                            gt = gap.tile([P, TS], f16, tag=f"ga{ib}",
                                          name=f"ga{ib}")
                            t0 = gstage.tile([P, TS], f16, tag="g0", name="g0")
                            t1 = gstage.tile([P, TS], f16, tag="g1", name="g1")
                            nc.sync.dma_start(t0[:], a2a_outs[gmt][gp])
                            nc.sync.dma_start(t1[:], a2a_outs[gmt][gp + 4])
                            nc.vector.tensor_add(gt[:], t0[:], t1[:])
                            ga[ib] = gt

        # ---- out-proj on my 512-token slice ----
        with tc.tile_pool(name="opsum", bufs=1, space="PSUM") as opsum, \
             tc.tile_pool(name="outsb", bufs=2) as outsb:
            pot = [opsum.tile([P, E], f32, tag=f"pot{tm}", name=f"pot{tm}")
                   for tm in range(TS // P)]
            for phase, ibs in ((0, [0, 2, 4, 6]), (1, [1, 3, 5, 7])):
                for tm in range(TS // P):
                    for n, ib in enumerate(ibs):
                        for oc in range(E // QW):
                            nc.tensor.matmul(
                                pot[tm][:, oc * QW:(oc + 1) * QW],
                                ga[ib][:, tm * P:(tm + 1) * P],
                                wo_r[:, ib * E + oc * QW:ib * E + oc * QW + QW],
                                start=(phase == 0 and n == 0), stop=False)
                    if phase == 1:
                        for oc in range(E // QW):
                            nc.tensor.matmul(
                                pot[tm][:, oc * QW:(oc + 1) * QW],
                                ones_b[:, 0:P],
                                bo_r[:, oc * QW:(oc + 1) * QW],
                                start=False, stop=True)
                        ot = outsb.tile([P, E], f32)
                        nc.scalar.activation(ot[:], pot[tm][:], AF.Copy)
                        for c in range(4):
                            nc.sync.dma_start(
                                out[tm * P + c * 32:tm * P + (c + 1) * 32, :],
                                ot[c * 32:(c + 1) * 32, :])

    nc.compile()
    return nc


def _get_nc():
    if 'nc' not in _cache:
        _cache['nc'] = _build()
    return _cache['nc']


def kernel(q, k, v, mask, Wq, bq, Wk, bk, Wv, bv, Wo, bo):
    from concourse.bass_utils import run_bass_kernel_spmd

    nc = _get_nc()
    f32 = np.float32
    f16 = np.float16
    q = np.asarray(q, f32)
    k = np.asarray(k, f32)
    v = np.asarray(v, f32)

    qT = [np.ascontiguousarray(q[b].T).astype(f16) for b in range(B)]
    kT = [np.ascontiguousarray(k[b].T).astype(f16) for b in range(B)]
    vT = [np.ascontiguousarray(v[b].T).astype(f16) for b in range(B)]
    WqT = np.asarray(Wq, f32).T.astype(f16)
    WkT = np.asarray(Wk, f32).T.astype(f16)
    WvT = np.asarray(Wv, f32).T.astype(f16)
    WoT = np.asarray(Wo, f32).T.astype(f16)
    bq = np.asarray(bq, f32).astype(f16)
    bk = np.asarray(bk, f32).astype(f16)
    bv = np.asarray(bv, f32).astype(f16)
    bo = np.asarray(bo, f32).astype(f16)
    onesv = np.ones((1, QW), f16)
    maskf = (np.asarray(mask) != 0).astype(f32)  # [B, S]

    in_maps = []
    for r in range(NCORES):
        b, g = r // G, r % G
        cols = slice(g * EG, (g + 1) * EG)
        m_pb = np.ascontiguousarray(maskf[b].reshape(KB, P).T)       # [128,16]
        m_rep = np.ascontiguousarray(np.repeat(m_pb, G, axis=1))     # [128,64]
        zm = np.zeros((P, NCORES), f32)
        zm[:, b * G:(b + 1) * G] = 1.0
        cpk = np.concatenate([onesv[0], bq[cols], bk[cols], bv[cols], bo],
                             axis=0)[None, :].astype(f16)
        mpk = np.concatenate([m_pb, m_rep, zm], axis=1).astype(f32)
        in_maps.append({
            "xqT": qT[b], "xkT": kT[b], "xvT": vT[b],
            "wqT": np.ascontiguousarray(WqT[:, cols]),
            "wkT": np.ascontiguousarray(WkT[:, cols]),
            "wvT": np.ascontiguousarray(WvT[:, cols]),
            "woT": WoT,
            "cpack": np.ascontiguousarray(cpk),
            "mpack": np.ascontiguousarray(mpk),
        })

    res = run_bass_kernel_spmd(nc, in_maps, core_ids=list(range(NCORES)),
                               **_cache.get('run_kwargs', {}))
    _cache['last_results'] = res

    full = np.empty((B, S, E), f32)
    for r in range(NCORES):
        b, g = r // G, r % G
        full[b, g * TS:(g + 1) * TS, :] = res.results[r]["out"]
    return full


# revision 20
# speedup vs baseline: 1.1032x; 1.1032x over previous
"""Trainium2 Bass kernel for batched multi-head attention (B=2, S=2048, E=1024, H=16).

Sharding: core r = 4*b + g handles batch b and head-group g (4 heads, 256 emb cols).
- QKV projections: tensor-parallel over head groups (each core computes its 256
  output cols from the full 1024-dim input, streamed in 128-row blocks).
- Attention: each core runs 4 heads over all 2048 queries. Scores kept transposed
  [kt, qt]; softmax normalization deferred via a mask/ones column appended to V.
  The two heads of a 128-partition tile are software-pipelined (scores run one
  key-block ahead of PV) so the PE never waits on the Exp activation, and
  K-block weight loads overlap the other head's matmuls (disjoint row groups).
- Out-proj: token-parallel. Contexts are exchanged with per-head 4-rank AllToAll
  within each batch's core group; each core then computes its 512-token slice.
The whole datapath runs in fp16 (full-rate PE streaming + fast weight load,
half DMA bytes); all matmul accumulation stays in fp32 PSUM.
"""

import sys

if '/opt/trn_rl_repo' not in sys.path:
    sys.path.insert(0, '/opt/trn_rl_repo')

import numpy as np

P = 128
B, S, E, H, DH = 2, 2048, 1024, 16, 64
NCORES = 8
G = 4                 # head groups == cores per batch
EG = E // G           # 256 emb cols per group
TS = S // G           # 512 tokens per core in out-proj
KB = S // P           # 16 key-token blocks
IB = E // P           # 8 contraction blocks of 128
QW = 512              # matmul moving free-dim chunk
HW = 1024             # query half width in attention
SCALE = DH ** -0.5

_cache = {}


def _build():
    import concourse.bass as bass
    import concourse.mybir as mybir
    import concourse.tile as tile
    from concourse import bacc
    from contextlib import ExitStack

    f32 = mybir.dt.float32
    f16 = mybir.dt.float16
    AF = mybir.ActivationFunctionType

    nc = bacc.Bacc("TRN2", target_bir_lowering=False, debug=False,
                   num_devices=NCORES)

    xqT = nc.dram_tensor("xqT", [E, S], f16, kind="ExternalInput").ap()
    xkT = nc.dram_tensor("xkT", [E, S], f16, kind="ExternalInput").ap()
    xvT = nc.dram_tensor("xvT", [E, S], f16, kind="ExternalInput").ap()
    wqT = nc.dram_tensor("wqT", [E, EG], f16, kind="ExternalInput").ap()
    wkT = nc.dram_tensor("wkT", [E, EG], f16, kind="ExternalInput").ap()
    wvT = nc.dram_tensor("wvT", [E, EG], f16, kind="ExternalInput").ap()
    woT = nc.dram_tensor("woT", [E, E], f16, kind="ExternalInput").ap()
    # packed constants: [ones(512) | bq(256) | bk(256) | bv(256) | bo(1024)]
    cpack = nc.dram_tensor("cpack", [1, QW + 3 * EG + E], f16,
                           kind="ExternalInput").ap()
    # packed f32 per-key constants: [mask_pb(16) | maskrep(64) | zmask(8)]
    mpack = nc.dram_tensor("mpack", [P, KB + KB * G + NCORES], f32,
                           kind="ExternalInput").ap()
    out = nc.dram_tensor("out", [TS, E], f32, kind="ExternalOutput").ap()

    a2a_ins = [nc.dram_tensor(f"a2a_in{mt}", [NCORES, P, TS], f16).ap()
               for mt in range(2)]
    a2a_outs = [nc.dram_tensor(f"a2a_out{mt}", [NCORES, P, TS], f16).ap()
                for mt in range(2)]
    groups = [list(range(NCORES))]

    with tile.TileContext(nc) as tc, ExitStack() as top:
        const = top.enter_context(tc.tile_pool(name="const", bufs=1))

        cpk = const.tile([1, QW + 3 * EG + E], f16)
        nc.sync.dma_start(cpk[:], cpack[:])
        ones_b = cpk[:, 0:QW]
        bq_r = cpk[:, QW:QW + EG]
        bk_r = cpk[:, QW + EG:QW + 2 * EG]
        bv_r = cpk[:, QW + 2 * EG:QW + 3 * EG]
        bo_r = cpk[:, QW + 3 * EG:QW + 3 * EG + E]
        mpk = const.tile([P, KB + KB * G + NCORES], f32)
        nc.sync.dma_start(mpk[:], mpack[:])
        mask_t = mpk[:, 0:KB]
        maskrep_t = mpk[:, KB:KB + KB * G]
        zmask_t = mpk[:, KB + KB * G:]

        # persistent projection outputs
        proj_sb = top.enter_context(tc.tile_pool(name="proj_sb", bufs=1))
        qpT = [proj_sb.tile([P, S], f16, tag=f"qpT{m}", name=f"qpT{m}")
               for m in range(2)]
        kpT = [proj_sb.tile([P, S], f16, tag=f"kpT{m}", name=f"kpT{m}")
               for m in range(2)]
        # vp tiles: per kt-block, [P, 4 heads x (64 vals + 1 mask col)]
        vp_sb = [proj_sb.tile([P, G * (DH + 1)], f16, tag=f"vp{m}", name=f"vp{m}")
                 for m in range(KB)]

        # ---- projection weights ----
        with tc.tile_pool(name="wqkv", bufs=1) as wqkv:
            w_r = {}
            for name, wap in (("v", wvT), ("k", wkT), ("q", wqT)):
                wr = wqkv.tile([P, IB * EG], f16, tag=f"w{name}r", name=f"w{name}r")
                for i in range(IB):
                    nc.sync.dma_start(wr[:, i * EG:(i + 1) * EG],
                                      wap[i * P:(i + 1) * P, :])
                w_r[name] = wr

            # ---- V projection, token-major: out [t, e] directly ----
            # stationary = x block [128 i-dims, 128 tokens], moving = Wv
            # [128 i-dims, 256 cols]; accumulate over the 8 i-blocks, add
            # bias, then mask-scale straight from PSUM into vp_sb.
            with tc.tile_pool(name="xall", bufs=1) as xall_p:
                xall = []
                for i in range(IB):
                    xr = xall_p.tile([P, S], f16, tag=f"xv{i}", name=f"xv{i}")
                    for c in range(2):
                        nc.sync.dma_start(
                            xr[:, c * HW:(c + 1) * HW],
                            xvT[i * P:(i + 1) * P, c * HW:(c + 1) * HW])
                    xall.append(xr)
                with tc.tile_pool(name="vpsum", bufs=4, space="PSUM") as vpsum:
                    for t in range(KB):
                        vps = vpsum.tile([P, EG], f32)
                        for i in range(IB):
                            nc.tensor.matmul(
                                vps[:], xall[i][:, t * P:(t + 1) * P],
                                w_r["v"][:, i * EG:(i + 1) * EG],
                                start=(i == 0), stop=False)
                        nc.tensor.matmul(
                            vps[:], ones_b[:, 0:P], bv_r[:],
                            start=False, stop=True)
                        dst3 = vp_sb[t].rearrange("p (h e) -> p h e", e=DH + 1)
                        nc.vector.tensor_scalar_mul(
                            dst3[:, :, 0:DH],
                            vps.rearrange("p (h e) -> p h e", e=DH),
                            mask_t[:, t:t + 1])
                        nc.vector.tensor_copy(
                            dst3[:, :, DH:DH + 1],
                            maskrep_t[:, t * G:(t + 1) * G]
                            .rearrange("p (h e) -> p h e", e=1))

            # ---- K and Q projections: out [e_sel, t] transposed ----
            with tc.tile_pool(name="xst", bufs=4) as xst, \
                 tc.tile_pool(name="kqpsum", bufs=1, space="PSUM") as kqpsum:
                for name, xap, brow, dsts in (("k", xkT, bk_r, kpT),
                                              ("q", xqT, bq_r, qpT)):
                    kqs = [kqpsum.tile([P, S], f32, tag=f"kqs{m}", name=f"kqs{m}")
                           for m in range(2)]
                    for i in range(IB):
                        xr = xst.tile([P, S], f16)
                        for c in range(2):
                            nc.sync.dma_start(
                                xr[:, c * HW:(c + 1) * HW],
                                xap[i * P:(i + 1) * P, c * HW:(c + 1) * HW])
                        for m in range(2):
                            for c in range(S // QW):
                                nc.tensor.matmul(
                                    kqs[m][:, c * QW:(c + 1) * QW],
                                    w_r[name][:, i * EG + m * P:i * EG + (m + 1) * P],
                                    xr[:, c * QW:(c + 1) * QW],
                                    start=(i == 0), stop=False)
                    for m in range(2):
                        for c in range(S // QW):
                            nc.tensor.matmul(
                                kqs[m][:, c * QW:(c + 1) * QW],
                                brow[:, m * P:(m + 1) * P],
                                ones_b[:],
                                start=False, stop=True)
                    for m in range(2):
                        nc.vector.tensor_copy(dsts[m][:], kqs[m][:])

        # ---- out-proj weights: loaded during attention ----
        wo_pool = top.enter_context(tc.tile_pool(name="wo", bufs=1))
        wo_r = wo_pool.tile([P, IB * E], f16)
        for i in range(IB):
            for c in range(2):
                nc.sync.dma_start(
                    wo_r[c * 64:(c + 1) * 64, i * E:(i + 1) * E],
                    woT[i * P + c * 64:i * P + (c + 1) * 64, :])

        # ga[ib] holds emb rows [ib*128, (ib+1)*128) of the concat context =
        # head pair ib%2 of group ib//2; gathered right after that pair's
        # AllToAll so the DMAs overlap the remaining attention work.
        gap = top.enter_context(tc.tile_pool(name="gap", bufs=1))
        gstage = top.enter_context(tc.tile_pool(name="gstage", bufs=4))
        ga = {}

        # ---- attention: head pair (2mt, 2mt+1), scores one j-block ahead ----
        # PSUM budget: sp0/sp1 [128,1024] (2 banks each) + pv0/pv1 [65,1024]
        # (2 banks each) = 8 banks, all at bufs=1.
        with tc.tile_pool(name="spsum", bufs=1, space="PSUM") as spsum, \
             tc.tile_pool(name="pvpsum", bufs=1, space="PSUM") as pvpsum, \
             tc.tile_pool(name="expp", bufs=3) as expp, \
             tc.tile_pool(name="normp", bufs=2) as normp, \
             tc.tile_pool(name="sendp", bufs=3) as sendp:
            for mt in range(2):
                for half in range(2):
                    q0 = half * HW
                    pv = [pvpsum.tile([DH + 1, HW], f32, tag=f"pv{hh}",
                                      name=f"pv{hh}") for hh in range(2)]
                    es_cur = [None, None]
                    es_nxt = [None, None]

                    def emit_scores(j, es_dst):
                        for hh in range(2):
                            po = hh * DH
                            sp = spsum.tile([P, HW], f32, tag=f"sp{hh}",
                                            name=f"sp{hh}")
                            for c in range(HW // QW):
                                nc.tensor.matmul(
                                    sp[:, c * QW:(c + 1) * QW],
                                    kpT[mt][po:po + DH, j * P:(j + 1) * P],
                                    qpT[mt][po:po + DH,
                                            q0 + c * QW:q0 + (c + 1) * QW],
                                    start=True, stop=True)
                            es = expp.tile([P, HW], f16, tag=f"es{hh}",
                                           name=f"es{hh}")
                            nc.scalar.activation(es[:], sp[:], AF.Exp,
                                                 scale=SCALE)
                            es_dst[hh] = es

                    emit_scores(0, es_cur)
                    for j in range(KB):
                        if j + 1 < KB:
                            emit_scores(j + 1, es_nxt)
                        for hh in range(2):
                            h = 2 * mt + hh
                            for c in range(HW // QW):
                                nc.tensor.matmul(
                                    pv[hh][:, c * QW:(c + 1) * QW],
                                    vp_sb[j][:, h * (DH + 1):(h + 1) * (DH + 1)],
                                    es_cur[hh][:, c * QW:(c + 1) * QW],
                                    start=(j == 0), stop=(j == KB - 1))
                        es_cur, es_nxt = es_nxt, es_cur

                    # normalize. The reciprocal runs on the ACT engine
                    # (table lookup, ~1us, plenty accurate for 2e-2) so the
                    # slow DVE reciprocal never gates pv reuse or the sends.
                    for hh in range(2):
                        h = 2 * mt + hh
                        rec = normp.tile([1, HW], f32, tag=f"rec{hh}",
                                         name=f"rec{hh}")
                        nc.scalar.add_instruction(mybir.InstActivation(
                            name=nc.get_next_instruction_name(),
                            func=AF.Reciprocal,
                            ins=[nc.scalar.lower_ap(pv[hh][DH:DH + 1, :]),
                                 mybir.ImmediateValue(dtype=f32, value=0.0),
                                 mybir.ImmediateValue(dtype=f32, value=1.0),
                                 mybir.ImmediateValue(dtype=f32, value=0.0)],
                            outs=[nc.scalar.lower_ap(rec[:])]))
                        recB = normp.tile([DH, HW], f32, tag=f"recB{hh}",
                                          name=f"recB{hh}")
                        nc.gpsimd.partition_broadcast(recB[:], rec[:])
                        ctxn = normp.tile([DH, HW], f16, tag=f"ctxn{hh}",
                                          name=f"ctxn{hh}")
                        nc.vector.tensor_mul(ctxn[:], pv[hh][0:DH, :], recB[:])
                        # slots: my-batch pair (s, s+4); zmask zeroes the
                        # cross-batch one so the receiver's pair-sum works
                        for jj in range(2):
                            shard = half * 2 + jj
                            for slot in (shard, shard + 4):
                                st = sendp.tile([DH, TS], f16)
                                nc.vector.tensor_scalar_mul(
                                    st[:], ctxn[:, jj * TS:(jj + 1) * TS],
                                    zmask_t[0:DH, slot:slot + 1])
                                nc.sync.dma_start(
                                    a2a_ins[mt][slot][hh * DH:(hh + 1) * DH, :],
                                    st[:])
                # one exchange for the whole pair (both heads stacked in the
                # slot payload, matching the ga[ib] row layout exactly)
                nc.gpsimd.collective_compute(
                    "AllToAll", mybir.AluOpType.bypass,
                    replica_groups=groups,
                    ins=[a2a_ins[mt][:]], outs=[a2a_outs[mt][:]])
                if mt == 1:
                    # gather both pairs now: pair-0's exchange finished long
                    # ago, and emitting gathers only after the final
                    # collective keeps its send descriptors unblocked
                    for gmt in range(2):
                        for gp in range(G):
                            ib = gp * 2 + gmt
                            gt = gap.tile([P, TS], f16, tag=f"ga{ib}",
                                          name=f"ga{ib}")
                            t0 = gstage.tile([P, TS], f16, tag="g0", name="g0")
                            t1 = gstage.tile([P, TS], f16, tag="g1", name="g1")
                            nc.sync.dma_start(t0[:], a2a_outs[gmt][gp])
                            nc.sync.dma_start(t1[:], a2a_outs[gmt][gp + 4])
                            nc.vector.tensor_add(gt[:], t0[:], t1[:])
                            ga[ib] = gt

        # ---- out-proj on my 512-token slice ----
        with tc.tile_pool(name="opsum", bufs=1, space="PSUM") as opsum, \
             tc.tile_pool(name="outsb", bufs=2) as outsb:
            pot = [opsum.tile([P, E], f32, tag=f"pot{tm}", name=f"pot{tm}")
                   for tm in range(TS // P)]
            for phase, ibs in ((0, [0, 2, 4, 6]), (1, [1, 3, 5, 7])):
                for tm in range(TS // P):
                    for n, ib in enumerate(ibs):
                        for oc in range(E // QW):
                            nc.tensor.matmul(
                                pot[tm][:, oc * QW:(oc + 1) * QW],
                                ga[ib][:, tm * P:(tm + 1) * P],
                                wo_r[:, ib * E + oc * QW:ib * E + oc * QW + QW],
                                start=(phase == 0 and n == 0), stop=False)
                    if phase == 1:
                        for oc in range(E // QW):
                            nc.tensor.matmul(
                                pot[tm][:, oc * QW:(oc + 1) * QW],
                                ones_b[:, 0:P],
                                bo_r[:, oc * QW:(oc + 1) * QW],
                                start=False, stop=True)
                        ot = outsb.tile([P, E], f32)
                        nc.scalar.activation(ot[:], pot[tm][:], AF.Copy)
                        for c in range(4):
                            nc.sync.dma_start(
                                out[tm * P + c * 32:tm * P + (c + 1) * 32, :],
                                ot[c * 32:(c + 1) * 32, :])

    nc.compile()
    return nc


def _get_nc():
    if 'nc' not in _cache:
        _cache['nc'] = _build()
    return _cache['nc']


def kernel(q, k, v, mask, Wq, bq, Wk, bk, Wv, bv, Wo, bo):
    from concourse.bass_utils import run_bass_kernel_spmd

    nc = _get_nc()
    f32 = np.float32
    f16 = np.float16
    q = np.asarray(q, f32)
    k = np.asarray(k, f32)
    v = np.asarray(v, f32)

    qT = [np.ascontiguousarray(q[b].T).astype(f16) for b in range(B)]
    kT = [np.ascontiguousarray(k[b].T).astype(f16) for b in range(B)]
    vT = [np.ascontiguousarray(v[b].T).astype(f16) for b in range(B)]
    WqT = np.asarray(Wq, f32).T.astype(f16)
    WkT = np.asarray(Wk, f32).T.astype(f16)
    WvT = np.asarray(Wv, f32).T.astype(f16)
    WoT = np.asarray(Wo, f32).T.astype(f16)
    bq = np.asarray(bq, f32).astype(f16)
    bk = np.asarray(bk, f32).astype(f16)
    bv = np.asarray(bv, f32).astype(f16)
    bo = np.asarray(bo, f32).astype(f16)
    onesv = np.ones((1, QW), f16)
    maskf = (np.asarray(mask) != 0).astype(f32)  # [B, S]

    in_maps = []
    for r in range(NCORES):
        b, g = r // G, r % G
        cols = slice(g * EG, (g + 1) * EG)
        m_pb = np.ascontiguousarray(maskf[b].reshape(KB, P).T)       # [128,16]
        m_rep = np.ascontiguousarray(np.repeat(m_pb, G, axis=1))     # [128,64]
        zm = np.zeros((P, NCORES), f32)
        zm[:, b * G:(b + 1) * G] = 1.0
        cpk = np.concatenate([onesv[0], bq[cols], bk[cols], bv[cols], bo],
                             axis=0)[None, :].astype(f16)
        mpk = np.concatenate([m_pb, m_rep, zm], axis=1).astype(f32)
        in_maps.append({
            "xqT": qT[b], "xkT": kT[b], "xvT": vT[b],
            "wqT": np.ascontiguousarray(WqT[:, cols]),
            "wkT": np.ascontiguousarray(WkT[:, cols]),
            "wvT": np.ascontiguousarray(WvT[:, cols]),
            "woT": WoT,
            "cpack": np.ascontiguousarray(cpk),
            "mpack": np.ascontiguousarray(mpk),
        })

    res = run_bass_kernel_spmd(nc, in_maps, core_ids=list(range(NCORES)),
                               **_cache.get('run_kwargs', {}))
    _cache['last_results'] = res

    full = np.empty((B, S, E), f32)
    for r in range(NCORES):
        b, g = r // G, r % G
        full[b, g * TS:(g + 1) * TS, :] = res.results[r]["out"]
    return full


# revision 21
# speedup vs baseline: 1.1151x; 1.0109x over previous
"""Trainium2 Bass kernel for batched multi-head attention (B=2, S=2048, E=1024, H=16).

Sharding: core r = 4*b + g handles batch b and head-group g (4 heads, 256 emb cols).
- QKV projections: tensor-parallel over head groups (each core computes its 256
  output cols from the full 1024-dim input, streamed in 128-row blocks).
- Attention: each core runs 4 heads over all 2048 queries. Scores kept transposed
  [kt, qt]; softmax normalization deferred via a mask/ones column appended to V.
  The two heads of a 128-partition tile are software-pipelined (scores run one
  key-block ahead of PV) so the PE never waits on the Exp activation, and
  K-block weight loads overlap the other head's matmuls (disjoint row groups).
- Out-proj: token-parallel. Contexts are exchanged with per-head 4-rank AllToAll
  within each batch's core group; each core then computes its 512-token slice.
The whole datapath runs in fp16 (full-rate PE streaming + fast weight load,
half DMA bytes); all matmul accumulation stays in fp32 PSUM.
"""

import sys

if '/opt/trn_rl_repo' not in sys.path:
    sys.path.insert(0, '/opt/trn_rl_repo')

import numpy as np

P = 128
B, S, E, H, DH = 2, 2048, 1024, 16, 64
NCORES = 8
G = 4                 # head groups == cores per batch
EG = E // G           # 256 emb cols per group
TS = S // G           # 512 tokens per core in out-proj
KB = S // P           # 16 key-token blocks
IB = E // P           # 8 contraction blocks of 128
QW = 512              # matmul moving free-dim chunk
HW = 1024             # query half width in attention
SCALE = DH ** -0.5

_cache = {}


def _build():
    import concourse.bass as bass
    import concourse.mybir as mybir
    import concourse.tile as tile
    from concourse import bacc
    from contextlib import ExitStack

    f32 = mybir.dt.float32
    f16 = mybir.dt.float16
    AF = mybir.ActivationFunctionType

    nc = bacc.Bacc("TRN2", target_bir_lowering=False, debug=False,
                   num_devices=NCORES)

    xqT = nc.dram_tensor("xqT", [E, S], f16, kind="ExternalInput").ap()
    xkT = nc.dram_tensor("xkT", [E, S], f16, kind="ExternalInput").ap()
    xvT = nc.dram_tensor("xvT", [E, S], f16, kind="ExternalInput").ap()
    wqT = nc.dram_tensor("wqT", [E, EG], f16, kind="ExternalInput").ap()
    wkT = nc.dram_tensor("wkT", [E, EG], f16, kind="ExternalInput").ap()
    wvT = nc.dram_tensor("wvT", [E, EG], f16, kind="ExternalInput").ap()
    woT = nc.dram_tensor("woT", [E, E], f16, kind="ExternalInput").ap()
    # packed constants: [ones(512) | bq(256) | bk(256) | bv(256) | bo(1024)]
    cpack = nc.dram_tensor("cpack", [1, QW + 3 * EG + E], f16,
                           kind="ExternalInput").ap()
    # packed f32 per-key constants: [mask_pb(16) | maskrep(64) | zmask(8)]
    mpack = nc.dram_tensor("mpack", [P, KB + KB * G + NCORES], f32,
                           kind="ExternalInput").ap()
    out = nc.dram_tensor("out", [TS, E], f32, kind="ExternalOutput").ap()

    a2a_ins = [nc.dram_tensor(f"a2a_in{mt}", [NCORES, P, TS], f16).ap()
               for mt in range(2)]
    a2a_outs = [nc.dram_tensor(f"a2a_out{mt}", [NCORES, P, TS], f16).ap()
                for mt in range(2)]
    groups = [list(range(NCORES))]

    with tile.TileContext(nc) as tc, ExitStack() as top:
        const = top.enter_context(tc.tile_pool(name="const", bufs=1))

        cpk = const.tile([1, QW + 3 * EG + E], f16)
        nc.sync.dma_start(cpk[:], cpack[:])
        ones_b = cpk[:, 0:QW]
        bq_r = cpk[:, QW:QW + EG]
        bk_r = cpk[:, QW + EG:QW + 2 * EG]
        bv_r = cpk[:, QW + 2 * EG:QW + 3 * EG]
        bo_r = cpk[:, QW + 3 * EG:QW + 3 * EG + E]
        mpk = const.tile([P, KB + KB * G + NCORES], f32)
        nc.sync.dma_start(mpk[:], mpack[:])
        mask_t = mpk[:, 0:KB]
        maskrep_t = mpk[:, KB:KB + KB * G]
        zmask_t = mpk[:, KB + KB * G:]

        # persistent projection outputs
        proj_sb = top.enter_context(tc.tile_pool(name="proj_sb", bufs=1))
        qpT = [proj_sb.tile([P, S], f16, tag=f"qpT{m}", name=f"qpT{m}")
               for m in range(2)]
        kpT = [proj_sb.tile([P, S], f16, tag=f"kpT{m}", name=f"kpT{m}")
               for m in range(2)]
        # vp tiles: per kt-block, [P, 4 heads x (64 vals + 1 mask col)]
        vp_sb = [proj_sb.tile([P, G * (DH + 1)], f16, tag=f"vp{m}", name=f"vp{m}")
                 for m in range(KB)]

        # ---- projection weights ----
        with tc.tile_pool(name="wqkv", bufs=1) as wqkv:
            w_r = {}
            for name, wap in (("v", wvT), ("k", wkT), ("q", wqT)):
                wr = wqkv.tile([P, IB * EG], f16, tag=f"w{name}r", name=f"w{name}r")
                for i in range(IB):
                    nc.sync.dma_start(wr[:, i * EG:(i + 1) * EG],
                                      wap[i * P:(i + 1) * P, :])
                w_r[name] = wr

            # ---- V projection, token-major: out [t, e] directly ----
            # stationary = x block [128 i-dims, 128 tokens], moving = Wv
            # [128 i-dims, 256 cols]; accumulate over the 8 i-blocks, add
            # bias, then mask-scale straight from PSUM into vp_sb.
            with tc.tile_pool(name="xall", bufs=1) as xall_p:
                xall = []
                for i in range(IB):
                    xr = xall_p.tile([P, S], f16, tag=f"xv{i}", name=f"xv{i}")
                    for c in range(2):
                        nc.sync.dma_start(
                            xr[:, c * HW:(c + 1) * HW],
                            xvT[i * P:(i + 1) * P, c * HW:(c + 1) * HW])
                    xall.append(xr)
                with tc.tile_pool(name="vpsum", bufs=4, space="PSUM") as vpsum:
                    for t in range(KB):
                        vps = vpsum.tile([P, EG], f32)
                        for i in range(IB):
                            nc.tensor.matmul(
                                vps[:], xall[i][:, t * P:(t + 1) * P],
                                w_r["v"][:, i * EG:(i + 1) * EG],
                                start=(i == 0), stop=False)
                        nc.tensor.matmul(
                            vps[:], ones_b[:, 0:P], bv_r[:],
                            start=False, stop=True)
                        dst3 = vp_sb[t].rearrange("p (h e) -> p h e", e=DH + 1)
                        nc.vector.tensor_scalar_mul(
                            dst3[:, :, 0:DH],
                            vps.rearrange("p (h e) -> p h e", e=DH),
                            mask_t[:, t:t + 1])
                        nc.vector.tensor_copy(
                            dst3[:, :, DH:DH + 1],
                            maskrep_t[:, t * G:(t + 1) * G]
                            .rearrange("p (h e) -> p h e", e=1))

            # ---- K and Q projections: out [e_sel, t] transposed ----
            with tc.tile_pool(name="xst", bufs=4) as xst, \
                 tc.tile_pool(name="kqpsum", bufs=1, space="PSUM") as kqpsum:
                for name, xap, brow, dsts in (("k", xkT, bk_r, kpT),
                                              ("q", xqT, bq_r, qpT)):
                    kqs = [kqpsum.tile([P, S], f32, tag=f"kqs{m}", name=f"kqs{m}")
                           for m in range(2)]
                    for i in range(IB):
                        xr = xst.tile([P, S], f16)
                        for c in range(2):
                            nc.sync.dma_start(
                                xr[:, c * HW:(c + 1) * HW],
                                xap[i * P:(i + 1) * P, c * HW:(c + 1) * HW])
                        for m in range(2):
                            for c in range(S // QW):
                                nc.tensor.matmul(
                                    kqs[m][:, c * QW:(c + 1) * QW],
                                    w_r[name][:, i * EG + m * P:i * EG + (m + 1) * P],
                                    xr[:, c * QW:(c + 1) * QW],
                                    start=(i == 0), stop=False)
                    for m in range(2):
                        for c in range(S // QW):
                            nc.tensor.matmul(
                                kqs[m][:, c * QW:(c + 1) * QW],
                                brow[:, m * P:(m + 1) * P],
                                ones_b[:],
                                start=False, stop=True)
                    for m in range(2):
                        nc.vector.tensor_copy(dsts[m][:], kqs[m][:])

        # ---- out-proj weights: loaded during attention ----
        wo_pool = top.enter_context(tc.tile_pool(name="wo", bufs=1))
        wo_r = wo_pool.tile([P, IB * E], f16)
        for i in range(IB):
            for c in range(2):
                nc.sync.dma_start(
                    wo_r[c * 64:(c + 1) * 64, i * E:(i + 1) * E],
                    woT[i * P + c * 64:i * P + (c + 1) * 64, :])

        # ga[ib] holds emb rows [ib*128, (ib+1)*128) of the concat context =
        # head pair ib%2 of group ib//2; gathered right after that pair's
        # AllToAll so the DMAs overlap the remaining attention work.
        gap = top.enter_context(tc.tile_pool(name="gap", bufs=1))
        gstage = top.enter_context(tc.tile_pool(name="gstage", bufs=4))
        ga = {}

        # ---- attention: head pair (2mt, 2mt+1), scores one j-block ahead ----
        # PSUM budget: sp0/sp1 [128,1024] (2 banks each) + pv0/pv1 [65,1024]
        # (2 banks each) = 8 banks, all at bufs=1.
        with tc.tile_pool(name="spsum", bufs=1, space="PSUM") as spsum, \
             tc.tile_pool(name="pvpsum", bufs=1, space="PSUM") as pvpsum, \
             tc.tile_pool(name="expp", bufs=3) as expp, \
             tc.tile_pool(name="normp", bufs=2) as normp, \
             tc.tile_pool(name="sendp", bufs=8) as sendp:
            for mt in range(2):
                for half in range(2):
                    q0 = half * HW
                    pv = [pvpsum.tile([DH + 1, HW], f32, tag=f"pv{hh}",
                                      name=f"pv{hh}") for hh in range(2)]
                    es_cur = [None, None]
                    es_nxt = [None, None]

                    def emit_scores(j, es_dst):
                        for hh in range(2):
                            po = hh * DH
                            sp = spsum.tile([P, HW], f32, tag=f"sp{hh}",
                                            name=f"sp{hh}")
                            for c in range(HW // QW):
                                nc.tensor.matmul(
                                    sp[:, c * QW:(c + 1) * QW],
                                    kpT[mt][po:po + DH, j * P:(j + 1) * P],
                                    qpT[mt][po:po + DH,
                                            q0 + c * QW:q0 + (c + 1) * QW],
                                    start=True, stop=True)
                            es = expp.tile([P, HW], f16, tag=f"es{hh}",
                                           name=f"es{hh}")
                            nc.scalar.activation(es[:], sp[:], AF.Exp,
                                                 scale=SCALE)
                            es_dst[hh] = es

                    emit_scores(0, es_cur)
                    for j in range(KB):
                        if j + 1 < KB:
                            emit_scores(j + 1, es_nxt)
                        for hh in range(2):
                            h = 2 * mt + hh
                            for c in range(HW // QW):
                                nc.tensor.matmul(
                                    pv[hh][:, c * QW:(c + 1) * QW],
                                    vp_sb[j][:, h * (DH + 1):(h + 1) * (DH + 1)],
                                    es_cur[hh][:, c * QW:(c + 1) * QW],
                                    start=(j == 0), stop=(j == KB - 1))
                        es_cur, es_nxt = es_nxt, es_cur

                    # normalize: recips on ACT (fast table lookup, fine
                    # for 2e-2), then both broadcasts, then both muls so the
                    # pv banks free before any send work queues on the DVE
                    rec = [normp.tile([1, HW], f32, tag=f"rec{hh}",
                                      name=f"rec{hh}") for hh in range(2)]
                    recB = [normp.tile([DH, HW], f32, tag=f"recB{hh}",
                                       name=f"recB{hh}") for hh in range(2)]
                    ctxn = [normp.tile([DH, HW], f16, tag=f"ctxn{hh}",
                                       name=f"ctxn{hh}") for hh in range(2)]
                    for hh in range(2):
                        nc.scalar.add_instruction(mybir.InstActivation(
                            name=nc.get_next_instruction_name(),
                            func=AF.Reciprocal,
                            ins=[nc.scalar.lower_ap(pv[hh][DH:DH + 1, :]),
                                 mybir.ImmediateValue(dtype=f32, value=0.0),
                                 mybir.ImmediateValue(dtype=f32, value=1.0),
                                 mybir.ImmediateValue(dtype=f32, value=0.0)],
                            outs=[nc.scalar.lower_ap(rec[hh][:])]))
                        nc.gpsimd.partition_broadcast(recB[hh][:], rec[hh][:])
                    for hh in range(2):
                        nc.vector.tensor_mul(ctxn[hh][:], pv[hh][0:DH, :],
                                             recB[hh][:])
                    # slots: my-batch pair (s, s+4); zmask zeroes the
                    # cross-batch one so the receiver's pair-sum works
                    for hh in range(2):
                        h = 2 * mt + hh
                        for jj in range(2):
                            shard = half * 2 + jj
                            for slot in (shard, shard + 4):
                                st = sendp.tile([DH, TS], f16)
                                nc.vector.tensor_scalar_mul(
                                    st[:], ctxn[hh][:, jj * TS:(jj + 1) * TS],
                                    zmask_t[0:DH, slot:slot + 1])
                                nc.sync.dma_start(
                                    a2a_ins[mt][slot][hh * DH:(hh + 1) * DH, :],
                                    st[:])
                # one exchange for the whole pair (both heads stacked in the
                # slot payload, matching the ga[ib] row layout exactly)
                nc.gpsimd.collective_compute(
                    "AllToAll", mybir.AluOpType.bypass,
                    replica_groups=groups,
                    ins=[a2a_ins[mt][:]], outs=[a2a_outs[mt][:]])
                if mt == 1:
                    # gather both pairs now: pair-0's exchange finished long
                    # ago, and emitting gathers only after the final
                    # collective keeps its send descriptors unblocked
                    for gmt in range(2):
                        for gp in range(G):
                            ib = gp * 2 + gmt
                            gt = gap.tile([P, TS], f16, tag=f"ga{ib}",
                                          name=f"ga{ib}")
                            t0 = gstage.tile([P, TS], f16, tag="g0", name="g0")
                            t1 = gstage.tile([P, TS], f16, tag="g1", name="g1")
                            nc.sync.dma_start(t0[:], a2a_outs[gmt][gp])
                            nc.sync.dma_start(t1[:], a2a_outs[gmt][gp + 4])
                            nc.vector.tensor_add(gt[:], t0[:], t1[:])
                            ga[ib] = gt

        # ---- out-proj on my 512-token slice ----
        with tc.tile_pool(name="opsum", bufs=1, space="PSUM") as opsum, \
             tc.tile_pool(name="outsb", bufs=2) as outsb:
            pot = [opsum.tile([P, E], f32, tag=f"pot{tm}", name=f"pot{tm}")
                   for tm in range(TS // P)]
            for phase, ibs in ((0, [0, 2, 4, 6]), (1, [1, 3, 5, 7])):
                for tm in range(TS // P):
                    for n, ib in enumerate(ibs):
                        for oc in range(E // QW):
                            nc.tensor.matmul(
                                pot[tm][:, oc * QW:(oc + 1) * QW],
                                ga[ib][:, tm * P:(tm + 1) * P],
                                wo_r[:, ib * E + oc * QW:ib * E + oc * QW + QW],
                                start=(phase == 0 and n == 0), stop=False)
                    if phase == 1:
                        for oc in range(E // QW):
                            nc.tensor.matmul(
                                pot[tm][:, oc * QW:(oc + 1) * QW],
                                ones_b[:, 0:P],
                                bo_r[:, oc * QW:(oc + 1) * QW],
                                start=False, stop=True)
                        ot = outsb.tile([P, E], f32)
                        nc.scalar.activation(ot[:], pot[tm][:], AF.Copy)
                        for c in range(4):
                            nc.sync.dma_start(
                                out[tm * P + c * 32:tm * P + (c + 1) * 32, :],
                                ot[c * 32:(c + 1) * 32, :])

    nc.compile()
    return nc


def _get_nc():
    if 'nc' not in _cache:
        _cache['nc'] = _build()
    return _cache['nc']


def kernel(q, k, v, mask, Wq, bq, Wk, bk, Wv, bv, Wo, bo):
    from concourse.bass_utils import run_bass_kernel_spmd

    nc = _get_nc()
    f32 = np.float32
    f16 = np.float16
    q = np.asarray(q, f32)
    k = np.asarray(k, f32)
    v = np.asarray(v, f32)

    qT = [np.ascontiguousarray(q[b].T).astype(f16) for b in range(B)]
    kT = [np.ascontiguousarray(k[b].T).astype(f16) for b in range(B)]
    vT = [np.ascontiguousarray(v[b].T).astype(f16) for b in range(B)]
    WqT = np.asarray(Wq, f32).T.astype(f16)
    WkT = np.asarray(Wk, f32).T.astype(f16)
    WvT = np.asarray(Wv, f32).T.astype(f16)
    WoT = np.asarray(Wo, f32).T.astype(f16)
    bq = np.asarray(bq, f32).astype(f16)
    bk = np.asarray(bk, f32).astype(f16)
    bv = np.asarray(bv, f32).astype(f16)
    bo = np.asarray(bo, f32).astype(f16)
    onesv = np.ones((1, QW), f16)
    maskf = (np.asarray(mask) != 0).astype(f32)  # [B, S]

    in_maps = []
    for r in range(NCORES):
        b, g = r // G, r % G
        cols = slice(g * EG, (g + 1) * EG)
        m_pb = np.ascontiguousarray(maskf[b].reshape(KB, P).T)       # [128,16]
        m_rep = np.ascontiguousarray(np.repeat(m_pb, G, axis=1))     # [128,64]
        zm = np.zeros((P, NCORES), f32)
        zm[:, b * G:(b + 1) * G] = 1.0
        cpk = np.concatenate([onesv[0], bq[cols], bk[cols], bv[cols], bo],
                             axis=0)[None, :].astype(f16)
        mpk = np.concatenate([m_pb, m_rep, zm], axis=1).astype(f32)
        in_maps.append({
            "xqT": qT[b], "xkT": kT[b], "xvT": vT[b],
            "wqT": np.ascontiguousarray(WqT[:, cols]),
            "wkT": np.ascontiguousarray(WkT[:, cols]),
            "wvT": np.ascontiguousarray(WvT[:, cols]),
            "woT": WoT,
            "cpack": np.ascontiguousarray(cpk),
            "mpack": np.ascontiguousarray(mpk),
        })

    res = run_bass_kernel_spmd(nc, in_maps, core_ids=list(range(NCORES)),
                               **_cache.get('run_kwargs', {}))
    _cache['last_results'] = res

    full = np.empty((B, S, E), f32)
    for r in range(NCORES):
        b, g = r // G, r % G
        full[b, g * TS:(g + 1) * TS, :] = res.results[r]["out"]
    return full


# revision 22
# speedup vs baseline: 1.1410x; 1.0232x over previous
"""Trainium2 Bass kernel for batched multi-head attention (B=2, S=2048, E=1024, H=16).

Sharding: core r = 4*b + g handles batch b and head-group g (4 heads, 256 emb cols).
- QKV projections: tensor-parallel over head groups (each core computes its 256
  output cols from the full 1024-dim input, streamed in 128-row blocks).
- Attention: each core runs 4 heads over all 2048 queries. Scores kept transposed
  [kt, qt]; softmax normalization deferred via a mask/ones column appended to V.
  The two heads of a 128-partition tile are software-pipelined (scores run one
  key-block ahead of PV) so the PE never waits on the Exp activation, and
  K-block weight loads overlap the other head's matmuls (disjoint row groups).
- Out-proj: token-parallel. Contexts are exchanged with per-head 4-rank AllToAll
  within each batch's core group; each core then computes its 512-token slice.
The whole datapath runs in fp16 (full-rate PE streaming + fast weight load,
half DMA bytes); all matmul accumulation stays in fp32 PSUM.
"""

import sys

if '/opt/trn_rl_repo' not in sys.path:
    sys.path.insert(0, '/opt/trn_rl_repo')

import numpy as np

P = 128
B, S, E, H, DH = 2, 2048, 1024, 16, 64
NCORES = 8
G = 4                 # head groups == cores per batch
EG = E // G           # 256 emb cols per group
TS = S // G           # 512 tokens per core in out-proj
KB = S // P           # 16 key-token blocks
IB = E // P           # 8 contraction blocks of 128
QW = 512              # matmul moving free-dim chunk
HW = 1024             # query half width in attention
SCALE = DH ** -0.5

_cache = {}


def _build():
    import concourse.bass as bass
    import concourse.mybir as mybir
    import concourse.tile as tile
    from concourse import bacc
    from contextlib import ExitStack

    f32 = mybir.dt.float32
    f16 = mybir.dt.float16
    AF = mybir.ActivationFunctionType

    nc = bacc.Bacc("TRN2", target_bir_lowering=False, debug=False,
                   num_devices=NCORES)

    xqT = nc.dram_tensor("xqT", [E, S], f16, kind="ExternalInput").ap()
    xkT = nc.dram_tensor("xkT", [E, S], f16, kind="ExternalInput").ap()
    xvT = nc.dram_tensor("xvT", [E, S], f16, kind="ExternalInput").ap()
    wqT = nc.dram_tensor("wqT", [E, EG], f16, kind="ExternalInput").ap()
    wkT = nc.dram_tensor("wkT", [E, EG], f16, kind="ExternalInput").ap()
    wvT = nc.dram_tensor("wvT", [E, EG], f16, kind="ExternalInput").ap()
    woT = nc.dram_tensor("woT", [E, E], f16, kind="ExternalInput").ap()
    # packed constants: [ones(512) | bq(256) | bk(256) | bv(256) | bo(1024)]
    cpack = nc.dram_tensor("cpack", [1, QW + 3 * EG + E], f16,
                           kind="ExternalInput").ap()
    # packed f32 per-key constants: [mask_pb(16) | maskrep(64) | zmask(8)]
    mpack = nc.dram_tensor("mpack", [P, KB + KB * G + NCORES], f32,
                           kind="ExternalInput").ap()
    out = nc.dram_tensor("out", [TS, E], f32, kind="ExternalOutput").ap()

    a2a_ins = [nc.dram_tensor(f"a2a_in{mt}", [NCORES, P, TS], f16).ap()
               for mt in range(2)]
    a2a_outs = [nc.dram_tensor(f"a2a_out{mt}", [NCORES, P, TS], f16).ap()
                for mt in range(2)]
    groups = [list(range(NCORES))]

    with tile.TileContext(nc) as tc, ExitStack() as top:
        const = top.enter_context(tc.tile_pool(name="const", bufs=1))

        cpk = const.tile([1, QW + 3 * EG + E], f16)
        nc.sync.dma_start(cpk[:], cpack[:])
        ones_b = cpk[:, 0:QW]
        bq_r = cpk[:, QW:QW + EG]
        bk_r = cpk[:, QW + EG:QW + 2 * EG]
        bv_r = cpk[:, QW + 2 * EG:QW + 3 * EG]
        bo_r = cpk[:, QW + 3 * EG:QW + 3 * EG + E]
        mpk = const.tile([P, KB + KB * G + NCORES], f32)
        nc.sync.dma_start(mpk[:], mpack[:])
        mask_t = mpk[:, 0:KB]
        maskrep_t = mpk[:, KB:KB + KB * G]
        zmask_t = mpk[:, KB + KB * G:]

        # persistent projection outputs
        proj_sb = top.enter_context(tc.tile_pool(name="proj_sb", bufs=1))
        qpT = [proj_sb.tile([P, S], f16, tag=f"qpT{m}", name=f"qpT{m}")
               for m in range(2)]
        kpT = [proj_sb.tile([P, S], f16, tag=f"kpT{m}", name=f"kpT{m}")
               for m in range(2)]
        # vp tiles: per kt-block, [P, 4 heads x (64 vals + 1 mask col)]
        vp_sb = [proj_sb.tile([P, G * (DH + 1)], f16, tag=f"vp{m}", name=f"vp{m}")
                 for m in range(KB)]

        # ---- projection weights ----
        with tc.tile_pool(name="wqkv", bufs=1) as wqkv:
            w_r = {}
            for name, wap in (("v", wvT), ("k", wkT), ("q", wqT)):
                wr = wqkv.tile([P, IB * EG], f16, tag=f"w{name}r", name=f"w{name}r")
                for i in range(IB):
                    nc.sync.dma_start(wr[:, i * EG:(i + 1) * EG],
                                      wap[i * P:(i + 1) * P, :])
                w_r[name] = wr

            # ---- V projection, token-major: out [t, e] directly ----
            # stationary = x block [128 i-dims, 128 tokens], moving = Wv
            # [128 i-dims, 256 cols]; accumulate over the 8 i-blocks, add
            # bias, then mask-scale straight from PSUM into vp_sb.
            with tc.tile_pool(name="xall", bufs=1) as xall_p:
                xall = [xall_p.tile([P, S], f16, tag=f"xv{i}", name=f"xv{i}")
                        for i in range(IB)]
                # chunk-0 of every block first: the t-major V matmuls need
                # all blocks' low columns before any high column
                for c in range(2):
                    for i in range(IB):
                        nc.sync.dma_start(
                            xall[i][:, c * HW:(c + 1) * HW],
                            xvT[i * P:(i + 1) * P, c * HW:(c + 1) * HW])
                with tc.tile_pool(name="vpsum", bufs=4, space="PSUM") as vpsum:
                    for t in range(KB):
                        vps = vpsum.tile([P, EG], f32)
                        for i in range(IB):
                            nc.tensor.matmul(
                                vps[:], xall[i][:, t * P:(t + 1) * P],
                                w_r["v"][:, i * EG:(i + 1) * EG],
                                start=(i == 0), stop=False)
                        nc.tensor.matmul(
                            vps[:], ones_b[:, 0:P], bv_r[:],
                            start=False, stop=True)
                        dst3 = vp_sb[t].rearrange("p (h e) -> p h e", e=DH + 1)
                        nc.vector.tensor_scalar_mul(
                            dst3[:, :, 0:DH],
                            vps.rearrange("p (h e) -> p h e", e=DH),
                            mask_t[:, t:t + 1])
                        nc.vector.tensor_copy(
                            dst3[:, :, DH:DH + 1],
                            maskrep_t[:, t * G:(t + 1) * G]
                            .rearrange("p (h e) -> p h e", e=1))

            # ---- K and Q projections: out [e_sel, t] transposed ----
            with tc.tile_pool(name="xst", bufs=4) as xst, \
                 tc.tile_pool(name="kqpsum", bufs=1, space="PSUM") as kqpsum:
                for name, xap, brow, dsts in (("k", xkT, bk_r, kpT),
                                              ("q", xqT, bq_r, qpT)):
                    kqs = [kqpsum.tile([P, S], f32, tag=f"kqs{m}", name=f"kqs{m}")
                           for m in range(2)]
                    for i in range(IB):
                        xr = xst.tile([P, S], f16)
                        for c in range(2):
                            nc.sync.dma_start(
                                xr[:, c * HW:(c + 1) * HW],
                                xap[i * P:(i + 1) * P, c * HW:(c + 1) * HW])
                        for m in range(2):
                            for c in range(S // QW):
                                nc.tensor.matmul(
                                    kqs[m][:, c * QW:(c + 1) * QW],
                                    w_r[name][:, i * EG + m * P:i * EG + (m + 1) * P],
                                    xr[:, c * QW:(c + 1) * QW],
                                    start=(i == 0), stop=False)
                    for m in range(2):
                        for c in range(S // QW):
                            nc.tensor.matmul(
                                kqs[m][:, c * QW:(c + 1) * QW],
                                brow[:, m * P:(m + 1) * P],
                                ones_b[:],
                                start=False, stop=True)
                    for m in range(2):
                        nc.vector.tensor_copy(dsts[m][:], kqs[m][:])

        # ---- out-proj weights: loaded during attention ----
        wo_pool = top.enter_context(tc.tile_pool(name="wo", bufs=1))
        wo_r = wo_pool.tile([P, IB * E], f16)
        for i in range(IB):
            for c in range(2):
                nc.sync.dma_start(
                    wo_r[c * 64:(c + 1) * 64, i * E:(i + 1) * E],
                    woT[i * P + c * 64:i * P + (c + 1) * 64, :])

        # ga[ib] holds emb rows [ib*128, (ib+1)*128) of the concat context =
        # head pair ib%2 of group ib//2; gathered right after that pair's
        # AllToAll so the DMAs overlap the remaining attention work.
        gap = top.enter_context(tc.tile_pool(name="gap", bufs=1))
        gstage = top.enter_context(tc.tile_pool(name="gstage", bufs=4))
        ga = {}

        # ---- attention: head pair (2mt, 2mt+1), scores one j-block ahead ----
        # PSUM budget: sp0/sp1 [128,1024] (2 banks each) + pv0/pv1 [65,1024]
        # (2 banks each) = 8 banks, all at bufs=1.
        with tc.tile_pool(name="spsum", bufs=1, space="PSUM") as spsum, \
             tc.tile_pool(name="pvpsum", bufs=1, space="PSUM") as pvpsum, \
             tc.tile_pool(name="expp", bufs=3) as expp, \
             tc.tile_pool(name="normp", bufs=2) as normp, \
             tc.tile_pool(name="sendp", bufs=8) as sendp:
            for mt in range(2):
                for half in range(2):
                    q0 = half * HW
                    pv = [pvpsum.tile([DH + 1, HW], f32, tag=f"pv{hh}",
                                      name=f"pv{hh}") for hh in range(2)]
                    es_cur = [None, None]
                    es_nxt = [None, None]

                    def emit_scores(j, es_dst):
                        for hh in range(2):
                            po = hh * DH
                            sp = spsum.tile([P, HW], f32, tag=f"sp{hh}",
                                            name=f"sp{hh}")
                            for c in range(HW // QW):
                                nc.tensor.matmul(
                                    sp[:, c * QW:(c + 1) * QW],
                                    kpT[mt][po:po + DH, j * P:(j + 1) * P],
                                    qpT[mt][po:po + DH,
                                            q0 + c * QW:q0 + (c + 1) * QW],
                                    start=True, stop=True)
                            es = expp.tile([P, HW], f16, tag=f"es{hh}",
                                           name=f"es{hh}")
                            nc.scalar.activation(es[:], sp[:], AF.Exp,
                                                 scale=SCALE)
                            es_dst[hh] = es

                    emit_scores(0, es_cur)
                    for j in range(KB):
                        if j + 1 < KB:
                            emit_scores(j + 1, es_nxt)
                        for hh in range(2):
                            h = 2 * mt + hh
                            for c in range(HW // QW):
                                nc.tensor.matmul(
                                    pv[hh][:, c * QW:(c + 1) * QW],
                                    vp_sb[j][:, h * (DH + 1):(h + 1) * (DH + 1)],
                                    es_cur[hh][:, c * QW:(c + 1) * QW],
                                    start=(j == 0), stop=(j == KB - 1))
                        es_cur, es_nxt = es_nxt, es_cur

                    # normalize: recips on ACT (fast table lookup, fine
                    # for 2e-2), then both broadcasts, then both muls so the
                    # pv banks free before any send work queues on the DVE
                    rec = [normp.tile([1, HW], f32, tag=f"rec{hh}",
                                      name=f"rec{hh}") for hh in range(2)]
                    recB = [normp.tile([DH, HW], f32, tag=f"recB{hh}",
                                       name=f"recB{hh}") for hh in range(2)]
                    ctxn = [normp.tile([DH, HW], f16, tag=f"ctxn{hh}",
                                       name=f"ctxn{hh}") for hh in range(2)]
                    for hh in range(2):
                        nc.scalar.add_instruction(mybir.InstActivation(
                            name=nc.get_next_instruction_name(),
                            func=AF.Reciprocal,
                            ins=[nc.scalar.lower_ap(pv[hh][DH:DH + 1, :]),
                                 mybir.ImmediateValue(dtype=f32, value=0.0),
                                 mybir.ImmediateValue(dtype=f32, value=1.0),
                                 mybir.ImmediateValue(dtype=f32, value=0.0)],
                            outs=[nc.scalar.lower_ap(rec[hh][:])]))
                        nc.gpsimd.partition_broadcast(recB[hh][:], rec[hh][:])
                    for hh in range(2):
                        nc.vector.tensor_mul(ctxn[hh][:], pv[hh][0:DH, :],
                                             recB[hh][:])
                    # slots: my-batch pair (s, s+4); zmask zeroes the
                    # cross-batch one so the receiver's pair-sum works
                    for hh in range(2):
                        h = 2 * mt + hh
                        for jj in range(2):
                            shard = half * 2 + jj
                            for slot in (shard, shard + 4):
                                st = sendp.tile([DH, TS], f16)
                                nc.vector.tensor_scalar_mul(
                                    st[:], ctxn[hh][:, jj * TS:(jj + 1) * TS],
                                    zmask_t[0:DH, slot:slot + 1])
                                nc.sync.dma_start(
                                    a2a_ins[mt][slot][hh * DH:(hh + 1) * DH, :],
                                    st[:])
                # one exchange for the whole pair (both heads stacked in the
                # slot payload, matching the ga[ib] row layout exactly)
                nc.gpsimd.collective_compute(
                    "AllToAll", mybir.AluOpType.bypass,
                    replica_groups=groups,
                    ins=[a2a_ins[mt][:]], outs=[a2a_outs[mt][:]])
                if mt == 1:
                    # gather both pairs now: pair-0's exchange finished long
                    # ago, and emitting gathers only after the final
                    # collective keeps its send descriptors unblocked
                    for gmt in range(2):
                        for gp in range(G):
                            ib = gp * 2 + gmt
                            gt = gap.tile([P, TS], f16, tag=f"ga{ib}",
                                          name=f"ga{ib}")
                            t0 = gstage.tile([P, TS], f16, tag="g0", name="g0")
                            t1 = gstage.tile([P, TS], f16, tag="g1", name="g1")
                            nc.sync.dma_start(t0[:], a2a_outs[gmt][gp])
                            nc.sync.dma_start(t1[:], a2a_outs[gmt][gp + 4])
                            nc.vector.tensor_add(gt[:], t0[:], t1[:])
                            ga[ib] = gt

        # ---- out-proj on my 512-token slice ----
        with tc.tile_pool(name="opsum", bufs=1, space="PSUM") as opsum, \
             tc.tile_pool(name="outsb", bufs=2) as outsb:
            pot = [opsum.tile([P, E], f32, tag=f"pot{tm}", name=f"pot{tm}")
                   for tm in range(TS // P)]
            for phase, ibs in ((0, [0, 2, 4, 6]), (1, [1, 3, 5, 7])):
                for tm in range(TS // P):
                    for n, ib in enumerate(ibs):
                        for oc in range(E // QW):
                            nc.tensor.matmul(
                                pot[tm][:, oc * QW:(oc + 1) * QW],
                                ga[ib][:, tm * P:(tm + 1) * P],
                                wo_r[:, ib * E + oc * QW:ib * E + oc * QW + QW],
                                start=(phase == 0 and n == 0), stop=False)
                    if phase == 1:
                        for oc in range(E // QW):
                            nc.tensor.matmul(
                                pot[tm][:, oc * QW:(oc + 1) * QW],
                                ones_b[:, 0:P],
                                bo_r[:, oc * QW:(oc + 1) * QW],
                                start=False, stop=True)
                        ot = outsb.tile([P, E], f32)
                        nc.scalar.activation(ot[:], pot[tm][:], AF.Copy)
                        for c in range(4):
                            nc.sync.dma_start(
                                out[tm * P + c * 32:tm * P + (c + 1) * 32, :],
                                ot[c * 32:(c + 1) * 32, :])

    nc.compile()
    return nc


def _get_nc():
    if 'nc' not in _cache:
        _cache['nc'] = _build()
    return _cache['nc']


def kernel(q, k, v, mask, Wq, bq, Wk, bk, Wv, bv, Wo, bo):
    from concourse.bass_utils import run_bass_kernel_spmd

    nc = _get_nc()
    f32 = np.float32
    f16 = np.float16
    q = np.asarray(q, f32)
    k = np.asarray(k, f32)
    v = np.asarray(v, f32)

    qT = [np.ascontiguousarray(q[b].T).astype(f16) for b in range(B)]
    kT = [np.ascontiguousarray(k[b].T).astype(f16) for b in range(B)]
    vT = [np.ascontiguousarray(v[b].T).astype(f16) for b in range(B)]
    WqT = np.asarray(Wq, f32).T.astype(f16)
    WkT = np.asarray(Wk, f32).T.astype(f16)
    WvT = np.asarray(Wv, f32).T.astype(f16)
    WoT = np.asarray(Wo, f32).T.astype(f16)
    bq = np.asarray(bq, f32).astype(f16)
    bk = np.asarray(bk, f32).astype(f16)
    bv = np.asarray(bv, f32).astype(f16)
    bo = np.asarray(bo, f32).astype(f16)
    onesv = np.ones((1, QW), f16)
    maskf = (np.asarray(mask) != 0).astype(f32)  # [B, S]

    in_maps = []
    for r in range(NCORES):
        b, g = r // G, r % G
        cols = slice(g * EG, (g + 1) * EG)
        m_pb = np.ascontiguousarray(maskf[b].reshape(KB, P).T)       # [128,16]
        m_rep = np.ascontiguousarray(np.repeat(m_pb, G, axis=1))     # [128,64]
        zm = np.zeros((P, NCORES), f32)
        zm[:, b * G:(b + 1) * G] = 1.0
        cpk = np.concatenate([onesv[0], bq[cols], bk[cols], bv[cols], bo],
                             axis=0)[None, :].astype(f16)
        mpk = np.concatenate([m_pb, m_rep, zm], axis=1).astype(f32)
        in_maps.append({
            "xqT": qT[b], "xkT": kT[b], "xvT": vT[b],
            "wqT": np.ascontiguousarray(WqT[:, cols]),
            "wkT": np.ascontiguousarray(WkT[:, cols]),
            "wvT": np.ascontiguousarray(WvT[:, cols]),
            "woT": WoT,
            "cpack": np.ascontiguousarray(cpk),
            "mpack": np.ascontiguousarray(mpk),
        })

    res = run_bass_kernel_spmd(nc, in_maps, core_ids=list(range(NCORES)),
                               **_cache.get('run_kwargs', {}))
    _cache['last_results'] = res

    full = np.empty((B, S, E), f32)
    for r in range(NCORES):
        b, g = r // G, r % G
        full[b, g * TS:(g + 1) * TS, :] = res.results[r]["out"]
    return full
